# revision 26
# baseline (speedup 1.0000x reference)
"""Causal attention (B=8, S=2048, D=H=768) on 8 trn2 NeuronCores.

Data-parallel over batch: core c computes batch c entirely on-chip, no
collectives.

Algebra: scores = (x Wq)(x Wk)^T = x (Wq Wk^T) x^T with M = Wq Wk^T
precomputed on host.  One on-device projection t = x M replaces both q and
k; the scores' k-side operand is raw x^T.

Precision scheme (HW-validated):
  - t = x M and scores = t x^T run in float32r: PE streams f32r at
    ~1 cycle/row for moving dims >= 256 (same speed as fp16) with ~2.2x
    better effective precision than fp16 (probe: 1.49e-4 vs 3.3e-4 rel on
    a K=768 matmul).  V projection also f32r.
  - exp weights, transposes, and attn@V run in fp16.
  - softmax stats in fp32.

Per-core pipeline:
  phase V:  V[s,h] = x^T-blocks (stationary) x Wv (moving), f32r -> fp16
  phase 1a: tT = M (stationary) x xT (moving), f32r, psum -> f32r SBUF
  phase 2, per 128-row q-tile (descending), software-pipelined one stage
    (scores of tile i emitted before transposes/attn@V of tile i-1):
      scores strip [q, k<=q] in <=512-col chunks; causal mask on the diag
      block; flash-style per-chunk rowmax + exp (frees PSUM banks early);
      global max via per-chunk max combine; per-chunk rescale alpha_c
      folded into the PE transpose as exp_block^T @ diag(alpha) (regular
      matmul, not transpose-mode); out = sum_k expT x V; scale by
      1/rowsum where rowsum = sum_c alpha_c * chunk_expsum_c.

Host side: shards x over batch, pre-transposes/tiles, computes M,
replicates weights, gathers per-core outputs.
"""

from contextlib import ExitStack

import numpy as np

import bass_rust
import concourse.mybir as mybir
import concourse.tile as tile
from concourse import bacc
from concourse.bass_utils import run_bass_kernel_spmd
from concourse.masks import make_causal_mask, make_identity

B, S, D, H = 8, 2048, 768, 768
N_CORES = 8
P = 128
DT = D // P   # 6 d-tiles
HT = H // P   # 6 h-tiles
ST = S // P   # 16 s-tiles

f32 = mybir.dt.float32
f32r = mybir.dt.float32r
f16 = mybir.dt.float16


def chunk_widths(cols):
    """Split cols into <=512 chunks, avoiding chunks <256 when possible
    (f32r matmul drops to 4 cycles/row below 256 moving columns)."""
    ws = []
    rem = cols
    while rem > 0:
        if rem == 640:
            w = 384
        elif rem >= 512:
            w = 512
        else:
            w = rem
        ws.append(w)
        rem -= w
    return ws


def build_nc():
    nc = bacc.Bacc(None)

    # inputs ship pre-tiled ([128 partitions, ...]) from the host
    xT_d = nc.declare_dram_parameter("xT", [DT, P, S], f32r, isOutput=False)
    m_d = nc.declare_dram_parameter("m", [P, DT, H], f32r, isOutput=False)
    x16_d = nc.declare_dram_parameter("x16", [8, P, DT, 256], f16, isOutput=False)
    wv_d = nc.declare_dram_parameter("wv", [2, P, DT, 384], f16, isOutput=False)
    out_d = nc.declare_dram_parameter("out", [S, H], f32, isOutput=True)

    with tile.TileContext(nc, pool_alloc_mode="queue") as tc, ExitStack() as ctx:
        persist = ctx.enter_context(tc.tile_pool(name="persist", bufs=1))
        xT = persist.tile([P, DT, S], f32r)    # 48KB/part
        tT = persist.tile([P, HT, S], f32r)    # 48KB/part
        V16 = persist.tile([P, ST, H], f16)    # 24KB/part
        ident16 = persist.tile([P, P], f16)
        cmask = persist.tile([P, P], f32)
        make_identity(nc, ident16)
        make_causal_mask(nc, cmask, mask_val=-1e10)

        m32 = persist.tile([P, DT, H], f32r)   # 18KB/part
        wv16 = persist.tile([P, 2, DT, 384], f16)  # 9KB/part (hc-major)
        x16 = persist.tile([P, 8, DT, 256], f16)   # 24KB/part
        # sync ring carries the V-phase-critical fp16 loads first; the bulk
        # f32r x^T + M transfers are dep-gated on the first V eviction so
        # they do not steal HBM bandwidth from the latency-critical start
        nc.sync.dma_start(out=wv16[:, 0, 0:3, :], in_=wv_d[0][:, 0:3, :])
        nc.sync.dma_start(out=x16[:, 0, :, :], in_=x16_d[0])
        nc.sync.dma_start(out=wv16[:, 0, 3:6, :], in_=wv_d[0][:, 3:6, :])
        nc.sync.dma_start(out=wv16[:, 1, 0:3, :], in_=wv_d[1][:, 0:3, :])
        nc.sync.dma_start(out=x16[:, 1, :, :], in_=x16_d[1])
        nc.sync.dma_start(out=wv16[:, 1, 3:6, :], in_=wv_d[1][:, 3:6, :])
        for sc in range(2, 8):
            nc.sync.dma_start(out=x16[:, sc, :, :], in_=x16_d[sc])

        psum = ctx.enter_context(tc.tile_pool(name="psum", bufs=8, space="PSUM"))

        # PE warmup during the initial DMA wait: ~4.5us of dummy matmuls
        # so the HAM clock gate reaches 8/8 before real work arrives
        warm = psum.tile([P, 512], f32, tag="ps", name="ps")
        NWARM = 40
        for k in range(NWARM):
            nc.tensor.matmul(warm[:, 0:P], ident16, ident16,
                             start=(k == 0), stop=(k == NWARM - 1))

        # ---- phase V: V = x^T-blocks x Wv (fp16) -------------------------
        bulk_anchor = None
        for sb in range(ST):
            sc, sti = sb // 2, sb % 2
            ps0 = psum.tile([P, 512], f32, tag="ps", name="ps")
            ps1 = psum.tile([P, 512], f32, tag="ps", name="ps")
            for dt_ in range(DT):
                nc.tensor.matmul(ps0[:, 0:384],
                                 x16[:, sc, dt_, sti * P:(sti + 1) * P],
                                 wv16[:, 0, dt_, :],
                                 start=(dt_ == 0), stop=(dt_ == DT - 1))
            for dt_ in range(DT):
                nc.tensor.matmul(ps1[:, 0:384],
                                 x16[:, sc, dt_, sti * P:(sti + 1) * P],
                                 wv16[:, 1, dt_, :],
                                 start=(dt_ == 0), stop=(dt_ == DT - 1))
            nc.scalar.copy(V16[:, sb, 0:384], ps0[:, 0:384])
            nc.scalar.copy(V16[:, sb, 384:768], ps1[:, 0:384])
            if sb == 0:
                x16probe = persist.tile([P, 1], f16)
                bulk_anchor = nc.vector.tensor_copy(
                    x16probe, x16[:, 0, 0, 0:1])
                # sync-ring pieces need no gate: the ring itself orders
                # them behind the x16 chunks
                for dt_ in (1, 3, 5):
                    nc.sync.dma_start(out=xT[:, dt_, :], in_=xT_d[dt_][:, :])
                dma = nc.gpsimd.dma_start(out=m32, in_=m_d[:, :, :])
                bass_rust.add_dep_helper(
                    dma.ins, bulk_anchor.ins, sync=True,
                    reason="M waits for early x16 landing")
                for dt_ in (0, 2, 4):
                    dma = nc.gpsimd.dma_start(out=xT[:, dt_, :],
                                              in_=xT_d[dt_][:, :])
                    bass_rust.add_dep_helper(
                        dma.ins, bulk_anchor.ins, sync=True,
                        reason="bulk x32 waits for early x16 landing")

        # ---- phase 1a: tT = M-blocks x xT (f32r) -------------------------
        for ht in range(HT):
            pss = [psum.tile([P, 512], f32, tag="ps", name="ps")
                   for _ in range(4)]
            for dt_ in range(DT):
                stat = m32[:, dt_, ht * P:(ht + 1) * P]
                for sc in range(4):
                    nc.tensor.matmul(
                        pss[sc], stat, xT[:, dt_, sc * 512:(sc + 1) * 512],
                        start=(dt_ == 0), stop=(dt_ == DT - 1))
            for sc in range(4):
                nc.scalar.copy(tT[:, ht, sc * 512:(sc + 1) * 512], pss[sc])

        # ---- phase 2: attention ------------------------------------------
        with tc.tile_pool(name="p2_exp", bufs=3) as exp_pool, \
             tc.tile_pool(name="p2_expT", bufs=2) as expT_pool, \
             tc.tile_pool(name="p2_D", bufs=2) as d_pool, \
             tc.tile_pool(name="p2_stat", bufs=4) as stat_pool, \
             tc.tile_pool(name="p2_out", bufs=2) as out_pool:
            ps_s_pool = ps_t_pool = ps_o_pool = psum

            def emit_scores(qt):
                """Scores chunks + flash per-chunk stats; returns state for
                the finish stage."""
                L = qt + 1
                cols = L * P
                ws = chunk_widths(cols)
                nch = len(ws)
                offs = [sum(ws[:i]) for i in range(nch)]
                pss = [ps_s_pool.tile([P, 512], f32, tag="ps", name="ps")
                       for _ in range(nch)]
                m4 = stat_pool.tile([P, 4], f32, tag="m4", name="m4")
                rs4 = stat_pool.tile([P, 4], f32, tag="rs4", name="rs4")
                exp16 = exp_pool.tile([P, S], f16, tag="exp16", name="exp16")
                for ci in reversed(range(nch)):
                    for ht in range(HT):
                        nc.tensor.matmul(
                            pss[ci][:, :ws[ci]],
                            tT[:, ht, qt * P:(qt + 1) * P],
                            xT[:, ht, offs[ci]:offs[ci] + ws[ci]],
                            start=(ht == 0), stop=(ht == HT - 1))
                    if ci == nch - 1:
                        # causal mask on the diagonal 128 cols (chunk tail)
                        wl = ws[-1]
                        nc.vector.tensor_add(
                            pss[-1][:, wl - P:wl], pss[-1][:, wl - P:wl],
                            cmask)
                    nc.vector.tensor_reduce(
                        m4[:, ci:ci + 1], pss[ci][:, :ws[ci]],
                        axis=mybir.AxisListType.X, op=mybir.AluOpType.max,
                        negate=True)
                    nc.scalar.activation(
                        exp16[:, offs[ci]:offs[ci] + ws[ci]],
                        pss[ci][:, :ws[ci]],
                        mybir.ActivationFunctionType.Exp,
                        bias=m4[:, ci:ci + 1], scale=1.0,
                        accum_out=rs4[:, ci:ci + 1])
                # combine: nm = -global_max; alpha_c = exp(m_c - m)
                nm = stat_pool.tile([P, 1], f32, tag="nm", name="nm")
                nc.vector.tensor_reduce(
                    nm, m4[:, :nch], axis=mybir.AxisListType.X,
                    op=mybir.AluOpType.min)
                al4 = stat_pool.tile([P, 4], f32, tag="al4", name="al4")
                nc.scalar.activation(
                    al4[:, :nch], m4[:, :nch],
                    mybir.ActivationFunctionType.Exp, bias=nm, scale=-1.0)
                # rowsum = sum_c alpha_c * rs_c ; rinv = 1/rowsum
                pr4 = stat_pool.tile([P, 4], f32, tag="pr4", name="pr4")
                rsum = stat_pool.tile([P, 1], f32, tag="rsum", name="rsum")
                nc.vector.tensor_mul(pr4[:, :nch], al4[:, :nch], rs4[:, :nch])
                nc.vector.tensor_reduce(
                    rsum, pr4[:, :nch], axis=mybir.AxisListType.X,
                    op=mybir.AluOpType.add)
                rinv = stat_pool.tile([P, 1], f32, tag="rinv", name="rinv")
                nc.vector.reciprocal(rinv, rsum)
                # D_c = diag(alpha_c) as fp16 for the transpose-matmul
                Dt = d_pool.tile([P, 4, P], f16, tag="Dt", name="Dt")
                for ci in range(nch):
                    nc.vector.tensor_scalar_mul(
                        Dt[:, ci, :], ident16, al4[:, ci:ci + 1])
                return (qt, ws, offs, exp16, Dt, rinv)

            def emit_finish(state):
                qt, ws, offs, exp16, Dt, rinv = state
                L = qt + 1
                # chunk index of each 128-col block j
                cof = []
                for j in range(L):
                    c = 0
                    while offs[c] + ws[c] <= j * P:
                        c += 1
                    cof.append(c)
                expT = expT_pool.tile([P, ST, P], f16, tag="expT", name="expT")
                # one PSUM bank as a 4-slot transpose ring
                pst4 = ps_t_pool.tile([P, 4, P], f32, tag="ps", name="ps")
                for j in range(L):
                    pst = pst4[:, j % 4, :]
                    # expT_j = exp_block_j^T @ diag(alpha_{c(j)})
                    nc.tensor.matmul(
                        pst, exp16[:, j * P:(j + 1) * P], Dt[:, cof[j], :],
                        start=True, stop=True)
                    if j % 2 == 0:
                        nc.vector.tensor_copy(expT[:, j, :], pst)
                    else:
                        nc.scalar.copy(expT[:, j, :], pst)
                pso0 = ps_o_pool.tile([P, 512], f32, tag="ps", name="ps")
                pso1 = ps_o_pool.tile([P, 512], f32, tag="ps", name="ps")
                for j in range(L):
                    stat = expT[:, j, :]
                    nc.tensor.matmul(pso0[:, 0:384], stat, V16[:, j, 0:384],
                                     start=(j == 0), stop=(j == L - 1))
                    nc.tensor.matmul(pso1[:, 0:384], stat, V16[:, j, 384:768],
                                     start=(j == 0), stop=(j == L - 1))
                out_sb = out_pool.tile([P, H], f32, tag="out_sb", name="out_sb")
                nc.scalar.mul(out_sb[:, 0:384], pso0[:, 0:384], rinv)
                nc.sync.dma_start(
                    out=out_d[qt * P:(qt + 1) * P, 0:384],
                    in_=out_sb[:, 0:384])
                nc.vector.tensor_scalar_mul(
                    out_sb[:, 384:768], pso1[:, 0:384], rinv)
                nc.sync.dma_start(
                    out=out_d[qt * P:(qt + 1) * P, 384:768],
                    in_=out_sb[:, 384:768])

            # descending q-tiles, software-pipelined: depth 1 for big
            # tiles, depth 2 once tiles get small (their PE work no longer
            # covers the stats-chain latency of the previous tile)
            pending = []
            for qt in range(ST - 1, -1, -1):
                depth = 1 if qt + 1 > 6 else 2
                pending.append(emit_scores(qt))
                while len(pending) > depth:
                    emit_finish(pending.pop(0))
            while pending:
                emit_finish(pending.pop(0))

    nc.finalize()
    return nc


_NC_CACHE = None


def _get_nc():
    global _NC_CACHE
    if _NC_CACHE is None:
        _NC_CACHE = build_nc()
    return _NC_CACHE


def _tile_rows(a):
    """[D, N] -> [128, D//128, N] (partition-major SBUF layout)."""
    d, n = a.shape
    return np.ascontiguousarray(a.reshape(d // P, P, n).transpose(1, 0, 2))


def make_in_maps(x, Wq, Wk, Wv):
    M = (Wq.astype(np.float64) @ Wk.astype(np.float64).T).astype(np.float32)
    m_t = _tile_rows(M)
    wv_t = _tile_rows(Wv.astype(np.float16))          # [128, 6, 768] fp16
    wv_t = np.ascontiguousarray(
        wv_t.reshape(P, DT, 2, 384).transpose(2, 0, 1, 3))  # [2, 128, 6, 384]
    in_maps = []
    for c in range(N_CORES):
        xTt = _tile_rows(np.ascontiguousarray(x[c].T))  # [128, 6, 2048]
        x16 = np.ascontiguousarray(
            xTt.astype(np.float16).reshape(P, DT, 8, 256)
            .transpose(2, 0, 1, 3))                     # [8, 128, 6, 256]
        xT = np.ascontiguousarray(xTt.transpose(1, 0, 2))  # [6, 128, 2048]
        in_maps.append({"xT": xT, "x16": x16, "m": m_t, "wv": wv_t})
    return in_maps


def kernel(x, Wq, Wk, Wv):
    x = np.asarray(x, dtype=np.float32)
    Wq = np.asarray(Wq, dtype=np.float32)
    Wk = np.asarray(Wk, dtype=np.float32)
    Wv = np.asarray(Wv, dtype=np.float32)

    nc = _get_nc()
    in_maps = make_in_maps(x, Wq, Wk, Wv)
    res = run_bass_kernel_spmd(nc, in_maps, list(range(N_CORES)))
    out = np.stack([res.results[c]["out"] for c in range(N_CORES)], axis=0)
    return out.astype(np.float32)
